# revision 15
# baseline (speedup 1.0000x reference)
"""Trainium2 Bass kernel for nn_AdaConvNeXt (moe_routing) — v5.

Data-parallel over batch (16 images/core). Major changes vs v3:
  - Token routing via gpsimd ap_gather (compile-time idx tables): FFN and
    fast path each run on their 392 gathered tokens only (no masks, no
    duplicated matmul work). Outputs stored in gathered order; host
    scatters them back and adds the residual x.
  - Depthwise 7x7 conv: 37 taps on TensorE as 19 fp8-DR pair passes per
    (group, half), 12 taps on DVE (bf16 2x) accumulated onto the drained y.
  - LN affine + biases host-folded into fp8 weights; LN stats via
    ones-matmuls; batched stat math per 2-image block; istd/nmi rows
    broadcast via DRAM round trip.
  - Residual add + scatter + f32 convert on host (device outputs bf16
    branch contributions only).
"""

import os
import numpy as np
import ml_dtypes

import concourse.bass as bass
import concourse.bacc as bacc
import concourse.mybir as mybir
import concourse.tile as tile
from concourse.bass_utils import run_bass_kernel_spmd

VP = mybir._bass_rust.VecI64Pair
BF16 = mybir.dt.bfloat16
FP8 = mybir.dt.float8e4
F32 = mybir.dt.float32
I16 = mybir.dt.int16
ADD = mybir.AluOpType.add
MULT = mybir.AluOpType.mult
AF = mybir.ActivationFunctionType
DRM = mybir.MatmulPerfMode.DoubleRow

N_CORES = 8
B, C, H, W = 128, 384, 28, 28
N = H * W            # 784
BL = B // N_CORES    # 16 images per core
NG = C // 128        # 3 channel groups
FG = (4 * C) // 128  # 12 ffn groups
HALF = N // 2        # 392
NI = 400             # gathered slots (392 real + 8 pad)
EPS = 1e-6
STAT_BLK = 2
HP, WP = 34, 36
PPITCH = NG * HP * WP


def _tap_split():
    """Split the 49 taps: 12 on DVE, 37 on PE as 19 DR pairs.

    DR pair offsets must be even (16-bit SBUF read granularity), so pairs
    are built from taps whose padded-plane offsets differ by an even
    amount. The last PE tap is paired with a zero-weight dummy at +2.
    """
    dve = [(dy, 3) for dy in range(-3, 4)]
    dve += [(dy, 2) for dy in range(-3, 2)]
    pairs = []
    for dy in range(-3, 4):
        pairs.append(((dy, -3), (dy, -1)))   # delta 2
        pairs.append(((dy, -2), (dy, 0)))    # delta 2
    for dy in (-3, -1, 1):
        pairs.append(((dy, 1), (dy + 1, 1)))  # delta WP (36)
    pairs.append(((2, 2), (3, 2)))            # delta 36
    pairs.append(((3, 1), None))              # zero-weight dummy at +2
    return dve, pairs


DVE_TAPS, PE_PAIRS = _tap_split()
NP_PE = len(PE_PAIRS)  # 19 DR passes


def _off(dy, dx):
    return (3 + dy) * WP + (3 + dx)


def cap(ap, aplist):
    c = ap.copy()
    c.ap = VP(aplist)
    return c


def build_bass(BL_, SD, S1, S2f, S2q, sim_safe=False, no_gather=False,
               stages=3):
    nc = bacc.Bacc(None, target_bir_lowering=False, debug=False)

    xpq_d = nc.declare_dram_parameter("xpq", [BL_, C, HP, WP], FP8, isOutput=False)
    xpb_d = nc.declare_dram_parameter("xpb", [BL_, C, HP, WP], BF16, isOutput=False)
    idx_d = nc.declare_dram_parameter("idxt", [2, BL_, 128, NI // 16], I16,
                                      isOutput=False)
    convdr_d = nc.declare_dram_parameter("convdr", [128, NG, NP_PE, 2, 128], FP8,
                                         isOutput=False)
    w1dr_d = nc.declare_dram_parameter("w1dr", [128, FG, 2, 128], FP8, isOutput=False)
    w1sg_d = nc.declare_dram_parameter("w1sg", [128, FG, 128], FP8, isOutput=False)
    w2fdr_d = nc.declare_dram_parameter("w2fdr", [128, NG, 6, 2, 128], FP8,
                                        isOutput=False)
    w2qdr_d = nc.declare_dram_parameter("w2qdr", [128, NG, 2, 128], FP8, isOutput=False)
    w2qsg_d = nc.declare_dram_parameter("w2qsg", [128, NG, 128], FP8, isOutput=False)
    # cvec cols: 0..2 dwb*SD | 3..14 c1 (gelu bias) | 15..17 c1out | 18..20 c2
    #            | 21.. DVE tap weights (21 + 3*ti + g)
    NCV = 21 + 3 * len(DVE_TAPS)
    cvec_d = nc.declare_dram_parameter("cvec", [128, NCV], F32, isOutput=False)
    out1_d = nc.declare_dram_parameter("out1", [BL_, C, HALF], BF16, isOutput=True)
    out2_d = nc.declare_dram_parameter("out2", [BL_, C, HALF], BF16, isOutput=True)

    from contextlib import ExitStack
    with ExitStack() as es:
        tc = es.enter_context(tile.TileContext(nc))
        pool = lambda name, bufs, **kw: es.enter_context(
            tc.tile_pool(name=name, bufs=bufs, **kw))
        cpool = pool("consts", 1)
        xq_pool = pool("xq", 3)
        xb_pool = pool("xb", 3)
        y_pool = pool("ybuf", 4)
        ysq_pool = pool("ysq", 2)
        tz_pool = pool("tz", 2)
        z_pool = pool("zbuf", 2)
        zg_pool = pool("zg", 4)
        gq_pool = pool("gq", 2)
        bc_pool = pool("bcast", 4)
        ix_pool = pool("ix", 4)
        rows_pool = pool("rows", 1)
        ox_pool = pool("ox", 6)
        dram_pool = pool("dscratch", 4, space=bass.MemorySpace.DRAM)
        py_pool = pool("py", 2, space=bass.MemorySpace.PSUM)
        pst_pool = pool("pst", 2, space=bass.MemorySpace.PSUM)
        ph_pool = pool("ph", 2, space=bass.MemorySpace.PSUM)
        pp_pool = pool("pp", 2, space=bass.MemorySpace.PSUM)

        # ---- constants into SBUF ----
        convdr_sb = cpool.tile([128, NG, NP_PE, 2, 128], FP8)
        for _g in range(NG):
            nc.sync.dma_start(convdr_sb[:, _g], convdr_d[:, _g])
        w1dr_sb = cpool.tile([128, FG, 2, 128], FP8)
        nc.scalar.dma_start(w1dr_sb[:], w1dr_d[:])
        w1sg_sb = cpool.tile([128, FG, 128], FP8)
        nc.scalar.dma_start(w1sg_sb[:], w1sg_d[:])
        w2fdr_sb = cpool.tile([128, NG, 6, 2, 128], FP8)
        nc.scalar.dma_start(w2fdr_sb[:], w2fdr_d[:])
        w2qdr_sb = cpool.tile([128, NG, 2, 128], FP8)
        nc.scalar.dma_start(w2qdr_sb[:], w2qdr_d[:])
        w2qsg_sb = cpool.tile([128, NG, 128], FP8)
        nc.scalar.dma_start(w2qsg_sb[:], w2qsg_d[:])
        cvec_sb = cpool.tile([128, NCV], F32)
        nc.scalar.dma_start(cvec_sb[:], cvec_d[:])

        ones_col = cpool.tile([128, 1], BF16)
        nc.vector.memset(ones_col[:], 1.0)
        eps_col = cpool.tile([33, 1], F32)
        nc.vector.memset(eps_col[:], float(SD) * float(SD) * EPS)

        n_blocks = (BL_ + STAT_BLK - 1) // STAT_BLK

        state = {}

        def phase1(blk):
            imgs = list(range(blk * STAT_BLK, min((blk + 1) * STAT_BLK, BL_)))
            nb = len(imgs)

            srow = rows_pool.tile([33, N], F32)  # mu
            qrow = rows_pool.tile([33, N], F32)  # E[y^2]
            if sim_safe:  # uninit-tracking only; HW never reads the junk rows
                nc.vector.memset(srow[:], 0.0)
                nc.vector.memset(qrow[:], 0.0)

            y_tiles, ix_tiles = {}, {}
            for ii, img in enumerate(imgs):
                xpq = xq_pool.tile([128, NG, HP, WP], FP8)
                nc.sync.dma_start(
                    out=xpq[:], in_=xpq_d[img].rearrange("(g c) h w -> c g h w", g=NG))
                xpb = xb_pool.tile([128, NG, HP, WP], BF16)
                nc.sync.dma_start(
                    out=xpb[:], in_=xpb_d[img].rearrange("(g c) h w -> c g h w", g=NG))
                ixs = []
                for br in range(2):
                    ixt = ix_pool.tile([128, NI // 16], I16, tag=f"ix{br}")
                    nc.sync.dma_start(out=ixt[:], in_=idx_d[br, img])
                    ixs.append(ixt)
                ix_tiles[img] = ixs

                y_bf = y_pool.tile([128, NG, N], BF16)
                y_tiles[img] = y_bf
                ysq = ysq_pool.tile([128, NG, N], BF16)
                for g in range(NG):
                    # PE: 19 fp8-DR pair passes per half
                    for h in range(2):
                        py = py_pool.tile([128, 512], F32, tag="py")
                        pyout = cap(py[:], [[512, 128], [W, 14], [1, W]])
                        base = g * HP * WP + h * 14 * WP
                        for k, (ta, tb) in enumerate(PE_PAIRS):
                            oa = _off(*ta)
                            dlt = (_off(*tb) - oa) if tb is not None else 2
                            rhs = cap(xpq[:, g, 0:14, 0:W],
                                      [[PPITCH, 128], [dlt, 2], [WP, 14], [1, W]])
                            rhs.offset = xpq[:].offset + base + oa
                            nc.tensor.matmul(
                                pyout, convdr_sb[:, g, k], rhs,
                                start=(k == 0), stop=(k == NP_PE - 1),
                                perf_mode=DRM, skip_group_check=True)
                        # drain: y_s = psum + SD*dw_b  (bf16)
                        nc.scalar.activation(
                            y_bf[:, g, h * HALF:(h + 1) * HALF], py[:, 0:HALF],
                            AF.Identity, bias=cvec_sb[:, g:g + 1], scale=1.0)
                    # DVE taps accumulate onto y
                    for ti, (dy, dx) in enumerate(DVE_TAPS):
                        nc.vector.scalar_tensor_tensor(
                            out=y_bf[:, g], in0=xpb[:, g, 3 + dy:31 + dy, 3 + dx:31 + dx],
                            scalar=cvec_sb[:, 21 + 3 * ti + g:22 + 3 * ti + g],
                            in1=y_bf[:, g], op0=MULT, op1=ADD)
                    # ysq = (y_s/SD)^2 = y_true^2  (bf16)
                    nc.scalar.activation(ysq[:, g], y_bf[:, g], AF.Square,
                                         scale=1.0 / SD)

                # ---- LN stats ----
                for lam in range(2):
                    cs = slice(HALF * lam, HALF * lam + HALF)
                    pst = pst_pool.tile([33, 512], F32, tag="pst")
                    for g in range(NG):
                        nc.tensor.matmul(
                            pst[0:1, 0:HALF], ones_col[:], y_bf[:, g, cs],
                            start=(g == 0), stop=(g == NG - 1),
                            skip_group_check=True)
                    for g in range(NG):
                        nc.tensor.matmul(
                            pst[32:33, 0:HALF], ones_col[:], ysq[:, g, cs],
                            start=(g == 0), stop=(g == NG - 1),
                            tile_position=(0, 32),
                            skip_group_check=True)
                    ps = 32 * ii
                    nc.vector.tensor_scalar(
                        out=srow[ps:ps + 1, cs], in0=pst[0:1, 0:HALF],
                        scalar1=1.0 / (SD * C), scalar2=None, op0=MULT)
                    nc.vector.tensor_scalar(
                        out=qrow[ps:ps + 1, cs], in0=pst[32:33, 0:HALF],
                        scalar1=1.0 / C, scalar2=None, op0=MULT)

            # ---- batched stat math: A = istd/SD (bf16), Bc = -mu*istd (bf16) ----
            np_ = 32 * (nb - 1) + 1
            musq = rows_pool.tile([33, N], F32, tag="rw1")
            nc.vector.tensor_tensor(out=musq[:np_], in0=srow[:np_], in1=srow[:np_],
                                    op=MULT)
            veps = rows_pool.tile([33, N], F32, tag="rw2")
            nc.vector.scalar_tensor_tensor(
                out=veps[:np_], in0=musq[:np_], scalar=-1.0, in1=qrow[:np_],
                op0=MULT, op1=ADD)
            sd_s = rows_pool.tile([33, N], F32, tag="rw1")
            nc.scalar.activation(sd_s[:np_], veps[:np_], AF.Sqrt, bias=eps_col[:np_],
                                 scale=float(SD) * float(SD))
            a_r = rows_pool.tile([33, N], F32, tag="rw2")
            with nc.allow_low_precision(reason="branch output is gamma-scaled"):
                nc.vector.reciprocal_approx_fast(out=a_r[:np_], in_=sd_s[:np_])
            a_rb = rows_pool.tile([33, N], BF16, tag="rw3")
            nc.vector.tensor_scalar(
                out=a_rb[:np_], in0=a_r[:np_], scalar1=1.0, scalar2=None, op0=MULT)
            b_r = rows_pool.tile([33, N], BF16, tag="rw4")
            nc.vector.scalar_tensor_tensor(
                out=b_r[:np_], in0=srow[:np_], scalar=-float(SD), in1=a_r[:np_],
                op0=MULT, op1=MULT)

            stat_dr = {}
            for ii, img in enumerate(imgs):
                ps = 32 * ii
                sc = dram_pool.tile([2, N], BF16, tag="sc", name=f"sc{blk}_{ii}")
                nc.sync.dma_start(out=sc[0:1, :], in_=a_rb[ps:ps + 1, :])
                nc.sync.dma_start(out=sc[1:2, :], in_=b_r[ps:ps + 1, :])
                stat_dr[img] = sc
            state[blk] = (imgs, y_tiles, ix_tiles, stat_dr)

        def phase2a(blk):
            imgs, y_tiles, ix_tiles, stat_dr = state[blk]
            zg = {}
            for img in imgs:
                y_bf = y_tiles[img]
                sc = stat_dr[img]
                a_b = bc_pool.tile([128, N], BF16, tag="ab")
                nc.gpsimd.dma_start(out=a_b[:], in_=sc[0:1, :].partition_broadcast(128))
                b_b = bc_pool.tile([128, N], BF16, tag="bb")
                nc.gpsimd.dma_start(out=b_b[:], in_=sc[1:2, :].partition_broadcast(128))

                z_tok = z_pool.tile([128, N, 4], FP8)
                if sim_safe:  # slot 3 is gathered but never consumed
                    nc.vector.memset(z_tok[:, :, 3], 0.0)
                for g in range(NG):
                    tz = tz_pool.tile([128, N], BF16, tag="tz")
                    nc.vector.tensor_tensor(out=tz[:], in0=y_bf[:, g], in1=a_b[:],
                                            op=MULT)
                    nc.vector.tensor_tensor(out=z_tok[:, :, g], in0=tz[:], in1=b_b[:],
                                            op=ADD)
                zs = []
                for br in range(2):
                    z_g = zg_pool.tile([128, NI, 4], FP8, tag=f"zg{br}")
                    if no_gather:
                        nc.vector.tensor_scalar(
                            out=z_g[:], in0=z_tok[:, 0:NI], scalar1=1.0,
                            scalar2=None, op0=MULT)
                    else:
                        nc.gpsimd.ap_gather(
                            z_g[:], z_tok[:], ix_tiles[img][br][:],
                            channels=128, num_elems=N, d=4, num_idxs=NI)
                    zs.append(z_g)
                zg[img] = zs
            state[blk] = (imgs, y_tiles, ix_tiles, stat_dr, zg)

        def phase2b(blk):
            imgs, y_tiles, ix_tiles, stat_dr, zg = state.pop(blk)
            for img in imgs:
                z_g1, z_g2 = zg[img]
                g_q = gq_pool.tile([128, FG, HALF], FP8)
                pp_f = []
                for fg in range(FG):
                    ph = ph_pool.tile([128, 512], F32, tag="ph")
                    rhs = cap(z_g1[:], [[NI * 4, 128], [2, 2], [4, HALF]])
                    nc.tensor.matmul(ph[:, 0:HALF], w1dr_sb[:, fg], rhs,
                                     start=True, stop=False,
                                     perf_mode=DRM, skip_group_check=True)
                    rhs = cap(z_g1[:], [[NI * 4, 128], [4, HALF]])
                    rhs.offset = z_g1[:].offset + 1
                    nc.tensor.matmul(ph[:, 0:HALF], w1sg_sb[:, fg], rhs,
                                     start=False, stop=True, skip_group_check=True)
                    nc.scalar.activation(
                        g_q[:, fg], ph[:, 0:HALF], AF.Gelu,
                        bias=cvec_sb[:, 3 + fg:4 + fg], scale=1.0 / S1)
                    # interleave fast-path passes to fill gelu-lag bubbles
                    if fg < NG:
                        og = fg
                        pq = pp_pool.tile([128, 512], F32, tag="pp", name=f"pq{og}")
                        rhs = cap(z_g2[:], [[NI * 4, 128], [2, 2], [4, HALF]])
                        nc.tensor.matmul(pq[:, 0:HALF], w2qdr_sb[:, og], rhs,
                                         start=True, stop=False,
                                         perf_mode=DRM, skip_group_check=True)
                        rhs = cap(z_g2[:], [[NI * 4, 128], [4, HALF]])
                        rhs.offset = z_g2[:].offset + 1
                        nc.tensor.matmul(pq[:, 0:HALF], w2qsg_sb[:, og], rhs,
                                         start=False, stop=True, skip_group_check=True)
                        ox2 = ox_pool.tile([128, HALF], BF16, tag="ox2")
                        nc.scalar.activation(
                            ox2[:], pq[:, 0:HALF], AF.Identity,
                            bias=cvec_sb[:, 18 + og:19 + og], scale=1.0 / S2q)
                        nc.gpsimd.dma_start(
                            out=out2_d[img, og * 128:(og + 1) * 128, :], in_=ox2[:])
                for og in range(NG):
                    pp = pp_pool.tile([128, 512], F32, tag="pp", name=f"pp{og}")
                    for j in range(6):
                        rhs = cap(g_q[:], [[FG * HALF, 128], [HALF, 2], [1, HALF]])
                        rhs.offset = g_q[:].offset + 2 * j * HALF
                        nc.tensor.matmul(pp[:, 0:HALF], w2fdr_sb[:, og, j], rhs,
                                         start=(j == 0), stop=(j == 5),
                                         perf_mode=DRM, skip_group_check=True)
                    ox1 = ox_pool.tile([128, HALF], BF16, tag="ox1")
                    nc.scalar.activation(
                        ox1[:], pp[:, 0:HALF], AF.Identity,
                        bias=cvec_sb[:, 15 + og:16 + og], scale=1.0 / S2f)
                    nc.gpsimd.dma_start(
                        out=out1_d[img, og * 128:(og + 1) * 128, :], in_=ox1[:])

        for step in range(n_blocks + 1):
            if step >= 1 and stages >= 2:
                phase2a(step - 1)
            if step < n_blocks:
                phase1(step)
            if step >= 1 and stages >= 3:
                phase2b(step - 1)
    nc.compile()
    return nc


# ---------------------------------------------------------------------------
# host side
# ---------------------------------------------------------------------------

def _pow2_scale(mat, target=64.0):
    m = float(np.abs(mat).max())
    if m == 0.0:
        return 1.0
    return float(2.0 ** np.floor(np.log2(target / m)))


def _fold_host(inputs):
    f32 = np.float32
    fp8 = ml_dtypes.float8_e4m3fn
    dw_w = np.asarray(inputs["dw_w"], f32)
    dw_b = np.asarray(inputs["dw_b"], f32)
    norm_w = np.asarray(inputs["norm_w"], f32)
    norm_b = np.asarray(inputs["norm_b"], f32)
    w1 = np.asarray(inputs["w1"], f32)
    b1 = np.asarray(inputs["b1"], f32)
    w2 = np.asarray(inputs["w2"], f32)
    b2 = np.asarray(inputs["b2"], f32)
    gamma = np.asarray(inputs["gamma"], f32)
    fp_norm_w = np.asarray(inputs["fp_norm_w"], f32)
    fp_norm_b = np.asarray(inputs["fp_norm_b"], f32)
    fp_w = np.asarray(inputs["fp_w"], f32)
    fp_b = np.asarray(inputs["fp_b"], f32)
    fp_gamma = np.asarray(inputs["fp_gamma"], f32)

    W1 = norm_w[:, None] * w1
    c1 = norm_b @ w1 + b1
    W2f = w2 * gamma[None, :]
    c1out = b2 * gamma
    W2q = (fp_norm_w[:, None] * fp_w) * fp_gamma[None, :]
    c2 = (fp_norm_b @ fp_w + fp_b) * fp_gamma

    SD = _pow2_scale(dw_w, 4.0)
    S1 = _pow2_scale(W1, 64.0)
    S2f = _pow2_scale(W2f, 64.0)
    S2q = _pow2_scale(W2q, 64.0)

    ar = np.arange(128)
    convdr = np.zeros((128, NG, NP_PE, 2, 128), f32)
    for g in range(NG):
        ch = slice(g * 128, (g + 1) * 128)
        for k, (ta, tb) in enumerate(PE_PAIRS):
            convdr[ar, g, k, 0, ar] = dw_w[ch, 0, ta[0] + 3, ta[1] + 3] * SD
            if tb is not None:
                convdr[ar, g, k, 1, ar] = dw_w[ch, 0, tb[0] + 3, tb[1] + 3] * SD
            # else: second row stays zero (dummy tap at +2)

    w1dr = np.zeros((128, FG, 2, 128), f32)
    w1sg = np.zeros((128, FG, 128), f32)
    for fg in range(FG):
        fs = slice(fg * 128, (fg + 1) * 128)
        w1dr[:, fg, 0] = W1[0:128, fs] * S1
        w1dr[:, fg, 1] = W1[256:384, fs] * S1
        w1sg[:, fg] = W1[128:256, fs] * S1
    w2fdr = np.zeros((128, NG, 6, 2, 128), f32)
    for og in range(NG):
        os_ = slice(og * 128, (og + 1) * 128)
        for j in range(6):
            w2fdr[:, og, j, 0] = W2f[(2 * j) * 128:(2 * j + 1) * 128, os_] * S2f
            w2fdr[:, og, j, 1] = W2f[(2 * j + 1) * 128:(2 * j + 2) * 128, os_] * S2f
    w2qdr = np.zeros((128, NG, 2, 128), f32)
    w2qsg = np.zeros((128, NG, 128), f32)
    for og in range(NG):
        os_ = slice(og * 128, (og + 1) * 128)
        w2qdr[:, og, 0] = W2q[0:128, os_] * S2q
        w2qdr[:, og, 1] = W2q[256:384, os_] * S2q
        w2qsg[:, og] = W2q[128:256, os_] * S2q

    NCV = 21 + 3 * len(DVE_TAPS)
    cvec = np.zeros((128, NCV), f32)
    for g in range(NG):
        cvec[:, g] = dw_b[g * 128:(g + 1) * 128] * SD
        for ti, (tdy, tdx) in enumerate(DVE_TAPS):
            cvec[:, 21 + 3 * ti + g] = dw_w[g * 128:(g + 1) * 128, 0,
                                            tdy + 3, tdx + 3] * SD
    for fg in range(FG):
        cvec[:, 3 + fg] = c1[fg * 128:(fg + 1) * 128]
    for og in range(NG):
        cvec[:, 15 + og] = c1out[og * 128:(og + 1) * 128]
        cvec[:, 18 + og] = c2[og * 128:(og + 1) * 128]

    return dict(
        convdr=convdr.astype(fp8),
        w1dr=w1dr.astype(fp8), w1sg=w1sg.astype(fp8),
        w2fdr=w2fdr.astype(fp8), w2qdr=w2qdr.astype(fp8),
        w2qsg=w2qsg.astype(fp8), cvec=cvec,
    ), SD, S1, S2f, S2q


def _idx_tables(idx1, idx2, Bn):
    """Wrapped int16 idx tables [2, Bn, 128, NI//16] for ap_gather."""
    out = np.zeros((2, Bn, 128, NI // 16), np.int16)
    pmod = np.arange(128) % 16
    cols = np.arange(NI // 16)
    slot = cols[None, :] * 16 + pmod[:, None]  # [128, NI//16]
    for br, idx in enumerate((idx1, idx2)):
        idx = np.asarray(idx, np.int64)
        for b in range(Bn):
            lst = np.zeros((NI,), np.int16)
            lst[:HALF] = idx[b]
            out[br, b] = lst[slot]
    return out


LAST_RESULT = None


def kernel(**inputs):
    global LAST_RESULT
    x = np.ascontiguousarray(np.asarray(inputs["x"], np.float32))
    Bn = x.shape[0]
    bl = Bn // N_CORES
    assert Bn % N_CORES == 0

    folded, SD, S1, S2f, S2q = _fold_host(inputs)
    idxt = _idx_tables(inputs["idx1"], inputs["idx2"], Bn)
    xpb = np.zeros((Bn, C, HP, WP), ml_dtypes.bfloat16)
    xpb[:, :, 3:31, 3:31] = x
    xpq = xpb.astype(ml_dtypes.float8_e4m3fn)

    nc = build_bass(bl, SD, S1, S2f, S2q)

    in_maps = []
    for c in range(N_CORES):
        sl = slice(c * bl, (c + 1) * bl)
        in_maps.append(dict(
            xpq=xpq[sl], xpb=xpb[sl], idxt=np.ascontiguousarray(idxt[:, sl]),
            **folded,
        ))

    trace = bool(int(os.environ.get("BASS_KERNEL_TRACE", "0")))
    res = run_bass_kernel_spmd(nc, in_maps, list(range(N_CORES)), trace=trace)
    LAST_RESULT = res
    o1 = np.concatenate([res.results[c]["out1"] for c in range(N_CORES)], axis=0)
    o2 = np.concatenate([res.results[c]["out2"] for c in range(N_CORES)], axis=0)

    # host: scatter branch outputs back to token positions, add residual
    out = np.zeros((Bn, C, N), np.float32)
    i1 = np.asarray(inputs["idx1"], np.int64)[:, None, :]
    i2 = np.asarray(inputs["idx2"], np.int64)[:, None, :]
    np.put_along_axis(out, np.broadcast_to(i1, (Bn, C, HALF)),
                      o1.astype(np.float32), axis=2)
    np.put_along_axis(out, np.broadcast_to(i2, (Bn, C, HALF)),
                      o2.astype(np.float32), axis=2)
    out += x.reshape(Bn, C, N)
    return out.reshape(Bn, C, H, W)
